# revision 4
# baseline (speedup 1.0000x reference)
"""DEMONetHashGraph Trainium2 kernel — 8-core data-parallel GNN (v2).

Strategy:
- Fold multi-hash einsum+concat+Wp into one [512,512] weight on host.
- Shard nodes (and their src-sorted edges) across 8 cores by range.
- Gather tables in fp8e4m3, chunk-major layout: the layer-1 table is built
  by 3 AllGathers (one Shared tensor per row-chunk), each issued as soon as
  layer-0 finishes the chunk's blocks, overlapping with remaining compute.
  Chunks < 32767 rows also keep gather indices within int16.
- Segment-sum via fp8 DoubleRow one-hot matmuls (one-hot matrices are
  precomputed on host and streamed from DRAM, not built on DVE).
- Dense matmuls bf16 (agg path + self path + bias), self rows pre-transposed
  on host (layer 0) / kept resident transposed (layer 1).
- ELU = max(x,0)-1 + exp(min(x,0)) with the exp chain on the Act engine.
- Tiny AllReduce for the per-graph pooled sums.
"""

import sys

for _p in ("/opt/trn_rl_repo", "/root/.axon_site/_ro/trn_rl_repo"):
    if _p not in sys.path:
        sys.path.insert(0, _p)

import time
from contextlib import ExitStack

import ml_dtypes
import numpy as np

import concourse.bass as bass
import concourse.mybir as mybir
import concourse.tile as tile
from concourse import bacc
from concourse.masks import make_identity

# problem constants (hardcoded per spec)
N_NODES = 50000
N_EDGES = 800000
D = 512
NUM_GRAPHS = 64
NUM_CLASSES = 10
NC = 8
SHARD = N_NODES // NC  # 6250
BN = 128
NB = (SHARD + BN - 1) // BN  # 49
SHARD_PAD = NB * BN  # 6272
NPAD = NC * SHARD_PAD  # 50176
# AllGather chunk boundaries in blocks; each chunk's table rows = 8*128*blocks
# must stay < 32767 for int16 gather indices
CHUNKS = (20, 40, NB)
NCH = len(CHUNKS)
CH_B0 = (0, *CHUNKS[:-1])
CH_ROWS = tuple((b1 - b0) * BN for b0, b1 in zip(CH_B0, CHUNKS))  # per-core rows
CH_BASE = tuple(NC * BN * b0 for b0 in CH_B0)  # global table base row
assert all(r * NC < 32768 for r in CH_ROWS)

f32 = mybir.dt.float32
f32r = mybir.dt.float32r
bf16 = mybir.dt.bfloat16
fp8 = mybir.dt.float8e4
i32 = mybir.dt.int32
i16 = mybir.dt.int16
BF = ml_dtypes.bfloat16
F8 = ml_dtypes.float8_e4m3


# table row for node n (chunk-major layout shared by x and h1 tables)
def _table_row(node):
    c = node // SHARD
    r = node % SHARD
    b = r // BN
    ch = np.zeros_like(b)
    for ci in range(NCH):
        ch = np.where((b >= CH_B0[ci]) & (b < CHUNKS[ci]), ci, ch)
    base = np.array(CH_BASE)[ch]
    rows = np.array(CH_ROWS)[ch]
    b0 = np.array(CH_B0)[ch]
    return base + c * rows + (r - b0 * BN)


def _preprocess(x, edge_index, batch, Hm1, Wp1, Ws1, b1, Hm2, Wp2, Ws2, b2, Wc, bc):
    x = np.asarray(x, np.float32)
    src = np.asarray(edge_index[0], np.int64)
    dst = np.asarray(edge_index[1], np.int64)
    batch = np.asarray(batch, np.int64)

    deg = np.bincount(src, minlength=N_NODES)
    iso = np.where(deg == 0)[0]
    if iso.size:
        src = np.concatenate([src, iso])
        dst = np.concatenate([dst, iso])

    order = np.argsort(src, kind="stable")
    src_s = src[order]
    dst_s = dst[order]
    dst_row = _table_row(dst_s)  # global table row (chunk-major)
    dst_ch = np.searchsorted(np.array(CH_BASE[1:] + (NPAD,)), dst_row, side="right")
    dst_loc = dst_row - np.array(CH_BASE)[dst_ch]  # row within chunk table

    # per-(core, block) edge ranges
    blk_starts = []
    for c in range(NC):
        for b in range(NB):
            blk_starts.append(c * SHARD + b * BN)
    blk_starts.append(N_NODES)
    bounds = np.searchsorted(src_s, np.array(blk_starts))

    # per-(core, block, chunk) counts -> per-(block, chunk) tile counts
    cnt = np.zeros((NC, NB, NCH), np.int64)
    for c in range(NC):
        for b in range(NB):
            k = c * NB + b
            ch = dst_ch[bounds[k] : bounds[k + 1]]
            for ci in range(NCH):
                cnt[c, b, ci] = int((ch == ci).sum())
    Tbc = (cnt.max(axis=0) + BN - 1) // BN  # [NB, NCH]
    Tb = Tbc.sum(axis=1)
    # pad so every block's total tile count is even (DoubleRow pairs)
    Tbc[:, -1] += Tb % 2
    Tb = Tbc.sum(axis=1)
    off = np.zeros(NB + 1, int)
    off[1:] = np.cumsum(Tb)
    offc = np.zeros((NCH, NB + 1), int)
    for ci in range(NCH):
        offc[ci, 1:] = np.cumsum(Tbc[:, ci])
    Tsum = int(off[-1])

    # one-hot (host-built, fp8) + idx arrays per chunk
    s_onehot = np.zeros((NC, BN, Tsum * BN), F8)
    idxs = [np.zeros((NC, 16, int(offc[ci, -1]) * 8), np.int16) for ci in range(NCH)]
    oh = np.zeros((BN, Tsum, BN), np.float32)
    for c in range(NC):
        oh[:] = 0.0
        for b in range(NB):
            k = c * NB + b
            e0, e1 = bounds[k], bounds[k + 1]
            ch_blk = dst_ch[e0:e1]
            d_blk = dst_loc[e0:e1]
            s_blk = (src_s[e0:e1] - (c * SHARD + b * BN)).astype(np.int64)
            t0 = off[b]
            for ci in range(NCH):
                sel = ch_blk == ci
                dsel = d_blk[sel]
                ssel = s_blk[sel]
                n = len(dsel)
                if n:
                    j = np.arange(n)
                    oh[j % BN, t0 + j // BN, ssel] = 1.0
                    idxs[ci][c, j % 16, offc[ci, b] * 8 + j // 16] = dsel.astype(
                        np.int16
                    )
                t0 += Tbc[b, ci]
        s_onehot[c] = oh.reshape(BN, Tsum * BN).astype(F8)

    # invdeg / batch slot per (core, partition, block)
    node_idx = (
        np.arange(NC)[:, None, None] * SHARD
        + np.arange(NB)[None, None, :] * BN
        + np.arange(BN)[None, :, None]
    )  # [NC, BN, NB]
    valid = node_idx < (np.arange(NC)[:, None, None] + 1) * SHARD
    node_clip = np.minimum(node_idx, N_NODES - 1)
    invdeg = np.where(valid, 1.0 / np.maximum(deg[node_clip], 1), 1.0).astype(
        np.float32
    )
    bslot = np.where(valid, batch[node_clip].astype(np.float32), -1.0).astype(
        np.float32
    )

    cnt_g = np.bincount(batch, minlength=NUM_GRAPHS).astype(np.float32)
    invcnt = np.broadcast_to(
        (1.0 / np.maximum(cnt_g, 1.0))[None, :], (BN, NUM_GRAPHS)
    ).copy()

    # fp8 gather table for x in chunk-major layout
    x_tab = np.zeros((NPAD, D), np.float32)
    x_tab[_table_row(np.arange(N_NODES))] = x
    x_f8 = x_tab.astype(F8)

    # host-transposed x self-path blocks: xT[c, p, (b,k,si)] = x[c*SHARD+b*128+si, k*128+p]
    x_pad = np.zeros((NC, SHARD_PAD, D), np.float32)
    x_pad[:, :SHARD, :] = x.reshape(NC, SHARD, D)
    xt = x_pad.reshape(NC, NB, BN, 4, BN).transpose(0, 4, 1, 3, 2)
    xT = np.ascontiguousarray(xt.reshape(NC, BN, NB * 4 * BN)).astype(BF)

    def fold(Hm, Wp):
        Hcat = np.concatenate([np.asarray(Hm, np.float32)[k] for k in range(4)], axis=1)
        return Hcat @ np.asarray(Wp, np.float32)

    w = dict(
        w1a=fold(Hm1, Wp1).astype(BF),
        wsa=np.asarray(Ws1, np.float32).astype(BF),
        w1b=fold(Hm2, Wp2).astype(BF),
        wsb=np.asarray(Ws2, np.float32).astype(BF),
        wc=np.asarray(Wc, np.float32),
        b1=np.asarray(b1, np.float32).reshape(1, D),
        b2=np.asarray(b2, np.float32).reshape(1, D),
        bc=np.asarray(bc, np.float32).reshape(1, NUM_CLASSES),
        ones=np.ones((1, BN), np.float32),
    )
    return dict(
        Tbc=tuple(tuple(int(v) for v in row) for row in Tbc),
        s_onehot=s_onehot,
        idxs=idxs,
        invdeg=invdeg,
        bslot=bslot,
        invcnt=invcnt,
        x_f8=x_f8,
        xT=xT,
        w=w,
    )


def _build(Tbc, reps=1, ablate=(), qmode=0):
    Tbc = np.array(Tbc)  # [NB, NCH]
    Tb = Tbc.sum(axis=1)
    off = np.zeros(NB + 1, int)
    off[1:] = np.cumsum(Tb)
    offc = np.zeros((NCH, NB + 1), int)
    for ci in range(NCH):
        offc[ci, 1:] = np.cumsum(Tbc[:, ci])
    Tsum = int(off[-1])
    Tmax = int(Tb.max())

    nc = bacc.Bacc(
        "TRN2",
        target_bir_lowering=False,
        debug=False,
        num_devices=NC,
        num_swdge_queues=2,
    )

    ein = dict(kind="ExternalInput")
    xg_d = nc.dram_tensor("xg", [NPAD, D], fp8, **ein)
    xT_d = nc.dram_tensor("xT", [BN, NB * 4 * BN], bf16, **ein)
    soh_d = nc.dram_tensor("soh", [BN, Tsum * BN], fp8, **ein)
    idx_d = [
        nc.dram_tensor(f"idx{ci}", [16, int(offc[ci, -1]) * 8], i16, **ein)
        for ci in range(NCH)
    ]
    invdeg_d = nc.dram_tensor("invdeg", [BN, NB], f32, **ein)
    bslot_d = nc.dram_tensor("bslot", [BN, NB], f32, **ein)
    invcnt_d = nc.dram_tensor("invcnt", [BN, NUM_GRAPHS], f32, **ein)
    w1a_d = nc.dram_tensor("w1a", [D, D], bf16, **ein)
    wsa_d = nc.dram_tensor("wsa", [D, D], bf16, **ein)
    w1b_d = nc.dram_tensor("w1b", [D, D], bf16, **ein)
    wsb_d = nc.dram_tensor("wsb", [D, D], bf16, **ein)
    wc_d = nc.dram_tensor("wc", [D, NUM_CLASSES], f32r, **ein)
    b1_d = nc.dram_tensor("b1", [1, D], f32r, **ein)
    b2_d = nc.dram_tensor("b2", [1, D], f32r, **ein)
    bc_d = nc.dram_tensor("bc", [1, NUM_CLASSES], f32r, **ein)
    ones_d = nc.dram_tensor("ones", [1, BN], f32r, **ein)
    out_d = nc.dram_tensor("out", [NUM_GRAPHS, NUM_CLASSES], f32, kind="ExternalOutput")

    with tile.TileContext(nc) as tc, ExitStack() as ctx:
        const = ctx.enter_context(tc.tile_pool(name="const", bufs=1))
        dram = ctx.enter_context(tc.tile_pool(name="dram", bufs=1, space="DRAM"))
        gpool = ctx.enter_context(tc.tile_pool(name="gpool", bufs=3))
        spool = ctx.enter_context(tc.tile_pool(name="spool", bufs=3))
        xpool = ctx.enter_context(tc.tile_pool(name="xpool", bufs=2))
        work = ctx.enter_context(tc.tile_pool(name="work", bufs=2))
        hpool = ctx.enter_context(tc.tile_pool(name="hpool", bufs=2))
        ps_nsum = ctx.enter_context(tc.tile_pool(name="ps_nsum", bufs=2, space="PSUM"))
        ps_tr = ctx.enter_context(tc.tile_pool(name="ps_tr", bufs=2, space="PSUM"))
        ps_dense = ctx.enter_context(
            tc.tile_pool(name="ps_dense", bufs=2, space="PSUM")
        )
        ps_pool = ctx.enter_context(tc.tile_pool(name="ps_pool", bufs=1, space="PSUM"))
        ps_fin = ctx.enter_context(tc.tile_pool(name="ps_fin", bufs=1, space="PSUM"))

        # ---- constants / loads ----
        ident_bf = const.tile([BN, BN], bf16)
        ident_f = const.tile([BN, BN], f32)
        make_identity(nc, ident_f[:])
        nc.vector.tensor_copy(ident_bf[:], ident_f[:])
        iota_i = const.tile([BN, BN], i32)
        nc.gpsimd.iota(iota_i[:], pattern=[[1, BN]], base=0, channel_multiplier=0)
        iota_g = const.tile([BN, NUM_GRAPHS], f32)
        nc.vector.tensor_copy(iota_g[:], iota_i[:, :NUM_GRAPHS])

        idx_sb = []
        for ci in range(NCH):
            t = const.tile([BN, int(offc[ci, -1]) * 8], i16, name=f"idx{ci}")
            for rep8 in range(8):
                nc.sync.dma_start(t[rep8 * 16 : (rep8 + 1) * 16, :], idx_d[ci][:, :])
            idx_sb.append(t)
        invdeg_sb = const.tile([BN, NB], f32)
        nc.sync.dma_start(invdeg_sb[:], invdeg_d[:, :])
        bslot_sb = const.tile([BN, NB], f32)
        nc.sync.dma_start(bslot_sb[:], bslot_d[:, :])
        invcnt_sb = const.tile([BN, NUM_GRAPHS], f32)
        nc.sync.dma_start(invcnt_sb[:], invcnt_d[:, :])

        def load_w(dram_t, dt=bf16):
            t = const.tile([BN, 4, D], dt, name=f"w_{dram_t.name}")
            nc.sync.dma_start(t[:], dram_t[:, :].rearrange("(ks kp) n -> kp ks n", kp=BN))
            return t

        w1a_sb = load_w(w1a_d)
        wsa_sb = load_w(wsa_d)
        w1b_sb = load_w(w1b_d)
        wsb_sb = load_w(wsb_d)
        wc_sb = const.tile([BN, 4, NUM_CLASSES], f32r)
        nc.sync.dma_start(wc_sb[:], wc_d[:, :].rearrange("(ks kp) n -> kp ks n", kp=BN))
        b1_sb = const.tile([1, D], f32r)
        nc.sync.dma_start(b1_sb[:], b1_d[:, :])
        b2_sb = const.tile([1, D], f32r)
        nc.sync.dma_start(b2_sb[:], b2_d[:, :])
        bc_sb = const.tile([1, NUM_CLASSES], f32r)
        nc.sync.dma_start(bc_sb[:], bc_d[:, :])
        ones_sb = const.tile([1, BN], f32r)
        nc.sync.dma_start(ones_sb[:], ones_d[:, :])

        # resident transposed h1 (self path of layer 1)
        hT1 = const.tile([BN, 4, SHARD_PAD], bf16)
        gacc = const.tile([BN, 4 * NUM_GRAPHS], f32)

        def layer(li, tables, w1_sb, ws_sb, bias_sb, h1s=None, h1f=None):
            for b in range(NB):
                tb = int(Tb[b])
                # gather neighbor rows (fp8), one gather per chunk table
                g = gpool.tile([BN, Tmax, D], fp8, name="g")
                t0 = 0
                for ci in range(NCH):
                    tci = int(Tbc[b, ci])
                    if tci == 0 or (int(offc[ci, b + 1]) - int(offc[ci, b])) == 0:
                        t0 += tci
                        continue
                    nidx = BN * (int(offc[ci, b + 1]) - int(offc[ci, b]))
                    nc.gpsimd.dma_gather(
                        g[:, t0 : t0 + tci, :],
                        tables[ci],
                        idx_sb[ci][:, offc[ci, b] * 8 : offc[ci, b + 1] * 8],
                        nidx,
                        nidx,
                        D,
                        single_packet=False,
                        queue_num=ci % 2,
                    )
                    t0 += tci
                # one-hot matrices (precomputed, streamed)
                s_t = spool.tile([BN, Tmax, BN], fp8, name="s_t")
                nc.sync.dma_start(
                    s_t[:, :tb, :],
                    soh_d[:, off[b] * BN : off[b + 1] * BN].rearrange(
                        "p (t j) -> p t j", j=BN
                    ),
                )
                # segment-sum into PSUM via DoubleRow fp8 one-hot matmuls
                ps = ps_nsum.tile([BN, D], f32, name="ps")
                npair = tb // 2
                if "edgemm" not in ablate:
                    for t in range(npair):
                        nc.tensor.matmul(
                            ps[:],
                            lhsT=s_t[:, 2 * t : 2 * t + 2, :],
                            rhs=g[:, 2 * t : 2 * t + 2, :],
                            start=(t == 0),
                            stop=(t == npair - 1),
                            perf_mode=mybir.MatmulPerfMode.DoubleRow,
                        )
                else:
                    nc.tensor.matmul(
                        ps[:],
                        lhsT=s_t[:, 0:2, :],
                        rhs=g[:, 0:2, :],
                        start=True,
                        stop=True,
                        perf_mode=mybir.MatmulPerfMode.DoubleRow,
                    )
                # mean (bf16 out) then transpose to [feat, node] quadrants
                agg = work.tile([BN, D], bf16, name="agg")
                nc.vector.tensor_scalar_mul(agg[:], ps[:], invdeg_sb[:, b : b + 1])
                pt = ps_tr.tile([BN, D], bf16, name="pt", tag="pt")
                for k in range(4):
                    nc.tensor.transpose(
                        pt[:, k * BN : (k + 1) * BN],
                        agg[:, k * BN : (k + 1) * BN],
                        ident_bf[:],
                    )
                aggT = work.tile([BN, 4, BN], bf16, name="aggT")
                nc.vector.tensor_copy(aggT[:], pt[:].rearrange("p (k n) -> p k n", n=BN))
                # self rows transposed
                if li == 0:
                    hbT = xpool.tile([BN, 4, BN], bf16, name="hbT")
                    nc.sync.dma_start(
                        hbT[:],
                        xT_d[:, b * 4 * BN : (b + 1) * 4 * BN].rearrange(
                            "p (k n) -> p k n", n=BN
                        ),
                    )
                else:
                    hbT = hT1[:, :, b * BN : (b + 1) * BN]
                # dense: po = bias + agg @ W1 + h @ Ws
                po = ps_dense.tile([BN, D], f32, name="po")
                nc.tensor.matmul(
                    po[:], lhsT=ones_sb[:, :], rhs=bias_sb[:, :], start=True, stop=False
                )
                for k in range(4):
                    nc.tensor.matmul(
                        po[:],
                        lhsT=aggT[:, k, :],
                        rhs=w1_sb[:, k, :],
                        start=False,
                        stop=False,
                    )
                for k in range(4):
                    nc.tensor.matmul(
                        po[:],
                        lhsT=hbT[:, k, :],
                        rhs=ws_sb[:, k, :],
                        start=False,
                        stop=(k == 3),
                    )
                # ELU: h = (max(x,0)-1) + exp(min(x,0));  exp chain on Act
                nrelu = work.tile([BN, D], f32, name="nrelu")
                nc.scalar.activation(
                    nrelu[:], po[:], mybir.ActivationFunctionType.Relu, scale=-1.0
                )
                e = work.tile([BN, D], f32, name="e")
                nc.scalar.activation(
                    e[:], nrelu[:], mybir.ActivationFunctionType.Exp, scale=-1.0
                )
                r = work.tile([BN, D], f32, name="r")
                nc.vector.tensor_scalar(
                    r[:], po[:], 0.0, -1.0, mybir.AluOpType.max, mybir.AluOpType.add
                )
                h = hpool.tile([BN, D], bf16, name="h")
                nc.vector.tensor_add(h[:], r[:], e[:])

                if li == 0:
                    # fp8 copy for the gathered table + AllGather input
                    h8 = work.tile([BN, D], fp8, name="h8")
                    nc.vector.tensor_copy(h8[:], h[:])
                    nc.sync.dma_start(h1s[b * BN : (b + 1) * BN, :], h8[:])
                    # resident transposed copy for layer-1 self path
                    pt3 = ps_tr.tile([BN, D], bf16, name="pt3", tag="pt")
                    for k in range(4):
                        nc.tensor.transpose(
                            pt3[:, k * BN : (k + 1) * BN],
                            h[:, k * BN : (k + 1) * BN],
                            ident_bf[:],
                        )
                    nc.vector.tensor_copy(
                        hT1[:, :, b * BN : (b + 1) * BN],
                        pt3[:].rearrange("p (k n) -> p k n", n=BN),
                    )
                    # AllGather chunk as soon as its rows are done
                    for ci in range(NCH):
                        if b == CHUNKS[ci] - 1:
                            nc.gpsimd.collective_compute(
                                "AllGather",
                                mybir.AluOpType.bypass,
                                replica_groups=[list(range(NC))],
                                ins=[h1s[CH_B0[ci] * BN : CHUNKS[ci] * BN, :]],
                                outs=[h1f[ci][:, :]],
                            )
                else:
                    # per-graph pooling: gacc += h.T @ onehot(batch)
                    bm = spool.tile([BN, NUM_GRAPHS], bf16, name="bm")
                    nc.vector.tensor_tensor(
                        out=bm[:],
                        in0=bslot_sb[:, b : b + 1].to_broadcast([BN, NUM_GRAPHS]),
                        in1=iota_g[:],
                        op=mybir.AluOpType.is_equal,
                    )
                    pg = ps_pool.tile([BN, 4 * NUM_GRAPHS], f32, name="pg")
                    for k in range(4):
                        nc.tensor.matmul(
                            pg[:, k * NUM_GRAPHS : (k + 1) * NUM_GRAPHS],
                            lhsT=h[:, k * BN : (k + 1) * BN],
                            rhs=bm[:],
                            start=True,
                            stop=True,
                        )
                    if b == 0:
                        nc.vector.tensor_copy(gacc[:], pg[:])
                    else:
                        nc.vector.tensor_add(gacc[:], gacc[:], pg[:])

        gin = dram.tile([BN, 4 * NUM_GRAPHS], f32)
        gout = dram.tile([BN, 4 * NUM_GRAPHS], f32, addr_space="Shared")

        for _rep in range(reps):
            h1s = dram.tile([SHARD_PAD, D], fp8, name=f"h1s_{_rep}")
            h1f = [
                dram.tile(
                    [NC * CH_ROWS[ci], D],
                    fp8,
                    addr_space="Shared",
                    name=f"h1f_{_rep}_{ci}",
                )
                for ci in range(NCH)
            ]
            xtabs = [
                xg_d[CH_BASE[ci] : CH_BASE[ci] + NC * CH_ROWS[ci], :]
                for ci in range(NCH)
            ]
            layer(0, xtabs, w1a_sb, wsa_sb, b1_sb, h1s=h1s, h1f=h1f)
            htabs = xtabs if "xgonly" in ablate else [t[:, :] for t in h1f]
            layer(1, htabs, w1b_sb, wsb_sb, b2_sb)

        # pooled sums all-reduce
        nc.sync.dma_start(gin[:, :], gacc[:])
        nc.gpsimd.collective_compute(
            "AllReduce",
            mybir.AluOpType.add,
            replica_groups=[list(range(NC))],
            ins=[gin[:, :]],
            outs=[gout[:, :]],
        )
        gsum = const.tile([BN, 4, NUM_GRAPHS], f32r)
        gs_raw = const.tile([BN, 4 * NUM_GRAPHS], f32)
        nc.sync.dma_start(gs_raw[:], gout[:, :])
        nc.vector.tensor_tensor(
            out=gsum[:],
            in0=gs_raw[:].rearrange("p (k g) -> p k g", g=NUM_GRAPHS),
            in1=invcnt_sb[:, None, :].to_broadcast([BN, 4, NUM_GRAPHS]),
            op=mybir.AluOpType.mult,
        )
        pf = ps_fin.tile([BN, NUM_CLASSES], f32)
        nc.tensor.matmul(
            pf[:NUM_GRAPHS, :],
            lhsT=ones_sb[:, :NUM_GRAPHS],
            rhs=bc_sb[:, :],
            start=True,
            stop=False,
        )
        for k in range(4):
            nc.tensor.matmul(
                pf[:NUM_GRAPHS, :],
                lhsT=gsum[:, k, :],
                rhs=wc_sb[:, k, :],
                start=False,
                stop=(k == 3),
            )
        o = const.tile([NUM_GRAPHS, NUM_CLASSES], f32)
        nc.vector.tensor_copy(o[:], pf[:NUM_GRAPHS, :])
        nc.sync.dma_start(out_d[:, :], o[:])

    nc.compile()
    return nc


def _make_in_maps(pre):
    w = pre["w"]
    in_maps = []
    for c in range(NC):
        m = {
            "xg": pre["x_f8"],
            "xT": np.ascontiguousarray(pre["xT"][c]),
            "soh": np.ascontiguousarray(pre["s_onehot"][c]),
            "invdeg": np.ascontiguousarray(pre["invdeg"][c]),
            "bslot": np.ascontiguousarray(pre["bslot"][c]),
            "invcnt": pre["invcnt"],
            "w1a": w["w1a"],
            "wsa": w["wsa"],
            "w1b": w["w1b"],
            "wsb": w["wsb"],
            "wc": w["wc"],
            "b1": w["b1"],
            "b2": w["b2"],
            "bc": w["bc"],
            "ones": w["ones"],
        }
        for ci in range(NCH):
            m[f"idx{ci}"] = np.ascontiguousarray(pre["idxs"][ci][c])
        in_maps.append(m)
    return in_maps


def _run_spmd(nc, in_maps, repeats=1):
    """Execute on 8 cores via PJRT (axon). Returns (out_core0, exec_times_s)."""
    import jax
    import jax.numpy as jnp  # noqa: F401
    from jax.sharding import Mesh, PartitionSpec, NamedSharding
    from jax.experimental.shard_map import shard_map

    import concourse.mybir as mb
    from concourse.bass2jax import (
        _bass_exec_p,
        install_neuronx_cc_hook,
        partition_id_tensor,
    )

    install_neuronx_cc_hook()
    partition_name = nc.partition_id_tensor.name if nc.partition_id_tensor else None

    in_names, out_names, out_avals, zero_outs = [], [], [], []
    for alloc in nc.m.functions[0].allocations:
        if not isinstance(alloc, mb.MemoryLocationSet):
            continue
        name = alloc.memorylocations[0].name
        if alloc.kind == "ExternalInput":
            if name != partition_name:
                in_names.append(name)
        elif alloc.kind == "ExternalOutput":
            shape = tuple(alloc.tensor_shape)
            dtype = mb.dt.np(alloc.dtype)
            out_names.append(name)
            out_avals.append(jax.core.ShapedArray(shape, dtype))
            zero_outs.append(np.zeros(shape, dtype))
    n_params = len(in_names)
    n_outs = len(out_avals)
    all_in_names = list(in_names) + out_names
    if partition_name is not None:
        all_in_names.append(partition_name)
    donate = tuple(range(n_params, n_params + n_outs))

    def _body(*args):
        operands = list(args)
        if partition_name is not None:
            operands.append(partition_id_tensor())
        outs = _bass_exec_p.bind(
            *operands,
            out_avals=tuple(out_avals),
            in_names=tuple(all_in_names),
            out_names=tuple(out_names),
            lowering_input_output_aliases=(),
            sim_require_finite=True,
            sim_require_nnan=True,
            nc=nc,
        )
        return tuple(outs)

    devices = jax.devices()[:NC]
    mesh = Mesh(np.asarray(devices), ("core",))
    in_specs = (PartitionSpec("core"),) * (n_params + n_outs)
    out_specs = (PartitionSpec("core"),) * len(out_names)
    sharded = jax.jit(
        shard_map(
            _body, mesh=mesh, in_specs=in_specs, out_specs=out_specs, check_rep=False
        ),
        donate_argnums=donate,
        keep_unused=True,
    )
    concat_in = [
        np.concatenate([np.asarray(in_maps[c][nm]) for c in range(NC)], axis=0)
        for nm in in_names
    ]
    shard_spec = NamedSharding(mesh, PartitionSpec("core"))
    concat_in_dev = [jax.device_put(a, shard_spec) for a in concat_in]

    def one_exec():
        zeros = [
            jax.device_put(
                np.zeros((NC * z.shape[0], *z.shape[1:]), z.dtype), shard_spec
            )
            for z in zero_outs
        ]
        t0 = time.perf_counter()
        out_arrs = sharded(*concat_in_dev, *zeros)
        jax.block_until_ready(out_arrs)
        return time.perf_counter() - t0, out_arrs

    times = []
    out_arrs = None
    for _ in range(max(1, repeats)):
        dt_s, out_arrs = one_exec()
        times.append(dt_s)

    outs0 = {
        name: np.asarray(out_arrs[i]).reshape(NC, *out_avals[i].shape)[0]
        for i, name in enumerate(out_names)
    }
    return outs0, times


_CACHE = {}


def _get_compiled(pre, reps=1, ablate=(), qmode=0):
    key = (pre["Tbc"], reps, tuple(ablate), qmode)
    if key not in _CACHE:
        _CACHE[key] = _build(pre["Tbc"], reps, ablate, qmode)
    return _CACHE[key]


def kernel(**inputs) -> np.ndarray:
    pre = _preprocess(**inputs)
    nc = _get_compiled(pre)
    outs, _ = _run_spmd(nc, _make_in_maps(pre), repeats=1)
    return outs["out"].astype(np.float32)


def kernel_timed(inputs, repeats=5, reps=1, ablate=()):
    pre = _preprocess(**inputs)
    nc = _get_compiled(pre, reps, ablate)
    outs, times = _run_spmd(nc, _make_in_maps(pre), repeats=repeats)
    return outs["out"].astype(np.float32), times
